# revision 10
# baseline (speedup 1.0000x reference)
"""Causal self-attention Trainium2 Bass kernel (v2).

Problem (hardcoded): B=4, S=2048, D=1024, H=16 heads, head_dim=64, fp32.
    qkv = x @ W_attn + b_attn; causal softmax attention; y @ W_proj + b_proj.

Sharding over 8 NeuronCores: core c -> (batch b = c//2, head-group g = c%2).
Each core computes, for its batch and its 8 heads (512 feature dims):
    Q^T, K^T [512f, 2048s] and V [2048s, 512f]  (fp32r, full-rate matmuls)
    flash-style causal attention in transposed layout:
        scores^T [128k, 512q] tiles = K^T.T @ Q^T   (per head, row-paired A/B)
        expS = exp(scores/8) via ACT (causal mask added in PSUM beforehand)
        out^T [64hd, 512q] += [V | ones].T @ expS   (denominator rides as row 64)
    normalization deferred + batched: denominators gathered per q-chunk into an
    [8, 512] tile (ACT copies), one reciprocal, K=1 PE broadcast, DVE multiply
    partial = y^T.T @ W_proj[group rows]  -> [2048, 1024]
Host: out[b] = partial(core 2b) + partial(core 2b+1) + b_proj + b_attn_v @ W_proj.
q/k biases are applied on-device (per-partition ACT bias); the v bias commutes
through softmax (rows sum to 1) so its projection is added on the host.
"""
import sys
if '/opt/trn_rl_repo' not in sys.path:
    sys.path.insert(0, '/opt/trn_rl_repo')

import numpy as np
import concourse.bass as bass
import concourse.mybir as mybir
import concourse.tile as tile
from concourse import bacc
from concourse import bass_utils

F32 = mybir.dt.float32
F32R = mybir.dt.float32r
AF = mybir.ActivationFunctionType
ALU = mybir.AluOpType

B, S, D, H, HD = 4, 2048, 1024, 16, 64
NCORES = 8
FPC = 512            # feature dims per core (8 heads * 64)
NPAIR = 4            # head pairs per core
DC = D // 128        # 8 contraction chunks for QKV/proj of x
NST = S // 128       # 16 s-tiles
MASKV = -30000.0     # exp(-30000/8) == 0 in fp32

_CACHE = {}


def _build_program():
    nc = bacc.Bacc("TRN2", target_bir_lowering=False, debug=False,
                   enable_asserts=False, num_devices=NCORES)

    xT_d = nc.dram_tensor("xT", [D, S], F32R, kind="ExternalInput").ap()
    wq_d = nc.dram_tensor("wq", [D, FPC], F32R, kind="ExternalInput").ap()
    wk_d = nc.dram_tensor("wk", [D, FPC], F32R, kind="ExternalInput").ap()
    wv_d = nc.dram_tensor("wv", [D, FPC], F32R, kind="ExternalInput").ap()
    wp_d = nc.dram_tensor("wp", [FPC, D], F32R, kind="ExternalInput").ap()
    bq_d = nc.dram_tensor("bq", [FPC], F32, kind="ExternalInput").ap()
    bk_d = nc.dram_tensor("bk", [FPC], F32, kind="ExternalInput").ap()
    out_d = nc.dram_tensor("out", [S, D], F32, kind="ExternalOutput").ap()

    from contextlib import ExitStack
    with tile.TileContext(nc) as tc, ExitStack() as ctx:
        persist = ctx.enter_context(tc.tile_pool(name="persist", bufs=1))
        QT = [persist.tile([128, S], F32R, name=f"qt{p}") for p in range(NPAIR)]
        KT = [persist.tile([128, S], F32R, name=f"kt{p}") for p in range(NPAIR)]
        # V tiles: [128 s, 8 heads, 65] -- col 64 is the ones column (denominator)
        Vt = [persist.tile([128, 8, 65], F32R, name=f"v{i}") for i in range(NST)]

        # ---------------- Phase 1: Q^T, K^T, V ----------------
        with ExitStack() as c1:
            wpool = c1.enter_context(tc.tile_pool(name="wpool", bufs=1))
            xpool = c1.enter_context(tc.tile_pool(name="xpool", bufs=2))
            p1ps = c1.enter_context(tc.tile_pool(name="p1ps", bufs=6, space="PSUM"))

            wq_sb = wpool.tile([128, DC, FPC], F32R, name="wq_sb")
            wk_sb = wpool.tile([128, DC, FPC], F32R, name="wk_sb")
            wv_sb = wpool.tile([128, DC, FPC], F32R, name="wv_sb")
            # per-chunk DMAs spread across queues (faster preamble)
            for c in range(DC):
                cs = slice(128 * c, 128 * c + 128)
                nc.sync.dma_start(wq_sb[:, c, :], wq_d[cs, :])
                nc.sync.dma_start(wk_sb[:, c, :], wk_d[cs, :])
                nc.sync.dma_start(wv_sb[:, c, :], wv_d[cs, :])
            bq_sb = wpool.tile([128, 4], F32, name="bq_sb")
            bk_sb = wpool.tile([128, 4], F32, name="bk_sb")
            nc.sync.dma_start(bq_sb[:], bq_d.rearrange("(c p) -> p c", p=128))
            nc.sync.dma_start(bk_sb[:], bk_d.rearrange("(c p) -> p c", p=128))

            onesv_f = wpool.tile([128, 8], F32, name="onesv_f")
            nc.gpsimd.memset(onesv_f[:], 1.0)
            for i in range(NST):
                nc.vector.tensor_copy(Vt[i][:, :, 64], onesv_f[:])

            for qtr in range(4):        # s-chunks of 512
                s0 = 512 * qtr
                xq = xpool.tile([128, DC, 512], F32R, name=f"xq{qtr}", tag="xq")
                for c in range(DC):
                    nc.sync.dma_start(xq[:, c, :],
                                      xT_d[128 * c:128 * c + 128, s0:s0 + 512])
                for f in range(4):      # feature chunks of 128 (= head pair)
                    for nm, w_sb, b_sb, dstT in (("q", wq_sb, bq_sb, QT),
                                                 ("k", wk_sb, bk_sb, KT)):
                        ps = p1ps.tile([128, 512], F32, name=f"ps{nm}{qtr}_{f}",
                                       tag="p1ps")
                        for c in range(DC):
                            nc.tensor.matmul(ps[:],
                                             w_sb[:, c, 128 * f:128 * f + 128],
                                             xq[:, c, :],
                                             start=(c == 0), stop=(c == DC - 1))
                        # psum->sbuf + per-feature bias on the idle ACT engine
                        nc.scalar.activation(dstT[f][:, s0:s0 + 512], ps[:],
                                             AF.Identity, bias=b_sb[:, f:f + 1])
                for ii in range(4):     # V s-tiles of 128 in this quarter
                    i = 4 * qtr + ii
                    psv = p1ps.tile([128, 512], F32, name=f"psv{i}", tag="p1ps")
                    for c in range(DC):
                        nc.tensor.matmul(psv[:],
                                         xq[:, c, 128 * ii:128 * ii + 128],
                                         wv_sb[:, c, :],
                                         start=(c == 0), stop=(c == DC - 1))
                    nc.vector.tensor_copy(
                        Vt[i][:, :, 0:64],
                        psv[:].rearrange("p (h u) -> p h u", h=8))

        # ---------------- Phase 2: attention + projection ----------------
        with ExitStack() as c2:
            per2 = c2.enter_context(tc.tile_pool(name="per2", bufs=1))
            expool = c2.enter_context(tc.tile_pool(name="expool", bufs=3))
            smpool = c2.enter_context(tc.tile_pool(name="smpool", bufs=2))
            outsb = c2.enter_context(tc.tile_pool(name="outsb", bufs=3))
            scps = c2.enter_context(tc.tile_pool(name="scps", bufs=2, space="PSUM"))
            pvps = c2.enter_context(tc.tile_pool(name="pvps", bufs=2, space="PSUM"))

            yT = [per2.tile([128, S], F32R, name=f"yt{p}") for p in range(NPAIR)]
            wp_sb = per2.tile([128, 4, D], F32R, name="wp_sb")
            for c in range(4):
                nc.sync.dma_start(wp_sb[:, c, :], wp_d[128 * c:128 * c + 128, :])

            # causal masks [128k, 1024] (A|B halves identical) for 4 diagonal
            # offsets: valid (keep 0) iff kp + 128*oi <= qf, else MASKV
            masks = [per2.tile([128, 1024], F32, name=f"mask{oi}")
                     for oi in range(4)]
            for oi in range(4):
                nc.gpsimd.memset(masks[oi][:], 0.0)
                for half in range(2):
                    nc.gpsimd.affine_select(
                        out=masks[oi][:, 512 * half:512 * half + 512],
                        in_=masks[oi][:, 512 * half:512 * half + 512],
                        compare_op=ALU.is_ge, fill=MASKV,
                        base=-128 * oi, pattern=[[1, 512]], channel_multiplier=-1)

            ones64_f = per2.tile([1, 64], F32, name="ones64_f")
            nc.gpsimd.memset(ones64_f[:], 1.0)
            ones64 = per2.tile([1, 64], F32R, name="ones64")
            nc.vector.tensor_copy(ones64[:], ones64_f[:])

            for j in range(4):          # q-chunks of 512
                q0 = 512 * j
                nk = 4 * (j + 1)
                for p in range(NPAIR):
                    accA = pvps.tile([65, 512], F32, name=f"accA{j}_{p}", tag="accA")
                    accB = pvps.tile([65, 512], F32, name=f"accB{j}_{p}", tag="accB")
                    for t in range(nk):
                        k0 = 128 * t
                        sc = scps.tile([128, 1024], F32, name=f"sc{j}_{p}_{t}",
                                       tag="sc")
                        nc.tensor.matmul(sc[:, 0:512],
                                         KT[p][0:64, k0:k0 + 128],
                                         QT[p][0:64, q0:q0 + 512],
                                         start=True, stop=True)
                        nc.tensor.matmul(sc[:, 512:1024],
                                         KT[p][64:128, k0:k0 + 128],
                                         QT[p][64:128, q0:q0 + 512],
                                         start=True, stop=True)
                        oi = t - 4 * j
                        if oi >= 0:     # diagonal tile -> causal mask (A+B at once)
                            nc.vector.tensor_tensor(sc[:], sc[:],
                                                    masks[oi][:], ALU.add)
                        ex = expool.tile([128, 1024], F32R, name=f"ex{j}_{p}_{t}",
                                         tag="ex")
                        nc.scalar.activation(ex[:], sc[:], AF.Exp, scale=0.125)
                        nc.tensor.matmul(accA[:], Vt[t][:, 2 * p, :],
                                         ex[:, 0:512],
                                         start=(t == 0), stop=(t == nk - 1))
                        nc.tensor.matmul(accB[:], Vt[t][:, 2 * p + 1, :],
                                         ex[:, 512:1024],
                                         start=(t == 0), stop=(t == nk - 1))
                    # normalize: fast reciprocal of denom row, K=1 PE broadcast,
                    # then in-place multiply of the unnormalized yT copy
                    for hi, acc in ((0, accA), (1, accB)):
                        r = 2 * p + hi
                        ys = yT[p][64 * hi:64 * hi + 64, q0:q0 + 512]
                        nc.vector.tensor_copy(ys, acc[0:64, :])
                        den = smpool.tile([1, 512], F32, name=f"den{j}_{r}",
                                          tag="den")
                        nc.scalar.copy(den[:], acc[64:65, :])
                        scr = smpool.tile([1, 512], F32, name=f"scr{j}_{r}",
                                          tag="scr")
                        rec = smpool.tile([1, 512], F32, name=f"rec{j}_{r}",
                                          tag="rec")
                        nc.vector.reciprocal_approx_accurate(
                            rec[:], den[:], scr[:])
                        rec_r = smpool.tile([1, 512], F32R, name=f"recr{j}_{r}",
                                            tag="rec_r")
                        nc.vector.tensor_copy(rec_r[:], rec[:])
                        bc = scps.tile([128, 1024], F32, name=f"bc{j}_{r}", tag="sc")
                        nc.tensor.matmul(bc[0:64, 0:512], ones64[:],
                                         rec_r[:], start=True, stop=True)
                        nc.vector.tensor_tensor(ys, bc[0:64, 0:512], ys, ALU.mult)
                # projection for this q-chunk (all pairs' yT just completed)
                for ii in range(4):
                    i = 4 * j + ii
                    for o in range(2):
                        po = scps.tile([128, 1024], F32, name=f"po{i}_{o}",
                                       tag="sc")
                        for p2 in range(NPAIR):
                            nc.tensor.matmul(po[:, 0:512],
                                             yT[p2][:, 128 * i:128 * i + 128],
                                             wp_sb[:, p2, 512 * o:512 * o + 512],
                                             start=(p2 == 0), stop=(p2 == 3))
                        ot = outsb.tile([128, 512], F32, name=f"ot{i}_{o}", tag="ot")
                        nc.vector.tensor_copy(ot[:], po[:, 0:512])
                        nc.sync.dma_start(
                            out_d[128 * i:128 * i + 128, 512 * o:512 * o + 512],
                            ot[:])

    nc.compile()
    return nc


def _get_program():
    if "nc" not in _CACHE:
        _CACHE["nc"] = _build_program()
    return _CACHE["nc"]


def kernel(x, W_attn, b_attn, W_proj, b_proj, _trace=False, _trace_cores=None):
    x = np.asarray(x, np.float32)
    W_attn = np.asarray(W_attn, np.float32)
    b_attn = np.asarray(b_attn, np.float32)
    W_proj = np.asarray(W_proj, np.float32)
    b_proj = np.asarray(b_proj, np.float32)

    nc = _get_program()

    in_maps = []
    for c in range(NCORES):
        b, g = divmod(c, 2)
        gc = slice(FPC * g, FPC * g + FPC)
        in_maps.append({
            "xT": np.ascontiguousarray(x[b].T),
            "wq": np.ascontiguousarray(W_attn[:, 0 * D:1 * D][:, gc]),
            "wk": np.ascontiguousarray(W_attn[:, 1 * D:2 * D][:, gc]),
            "wv": np.ascontiguousarray(W_attn[:, 2 * D:3 * D][:, gc]),
            "wp": np.ascontiguousarray(W_proj[gc, :]),
            "bq": np.ascontiguousarray(b_attn[0 * D:1 * D][gc]),
            "bk": np.ascontiguousarray(b_attn[1 * D:2 * D][gc]),
        })

    kw = {}
    if _trace:
        kw = dict(trace=True, trace_cores=_trace_cores or [0])
    res = bass_utils.run_bass_kernel_spmd(nc, in_maps, core_ids=list(range(NCORES)),
                                          **kw)

    # host-side reduction: v-bias commutes through softmax -> fold via W_proj
    corr = b_proj + b_attn[2 * D:3 * D] @ W_proj
    out = np.empty((B, S, D), np.float32)
    for b in range(B):
        out[b] = res.results[2 * b]["out"] + res.results[2 * b + 1]["out"] + corr

    if _trace:
        kernel._last_results = res
    return out


# revision 13
# speedup vs baseline: 1.1493x; 1.1493x over previous
"""Causal self-attention Trainium2 Bass kernel (v2).

Problem (hardcoded): B=4, S=2048, D=1024, H=16 heads, head_dim=64, fp32.
    qkv = x @ W_attn + b_attn; causal softmax attention; y @ W_proj + b_proj.

Sharding over 8 NeuronCores: core c -> (batch b = c//2, head-group g = c%2).
Each core computes, for its batch and its 8 heads (512 feature dims):
    Q^T, K^T [512f, 2048s] and V [2048s, 512f]  (fp32r, full-rate matmuls)
    flash-style causal attention in transposed layout:
        scores^T [128k, 512q] tiles = K^T.T @ Q^T   (per head, row-paired A/B)
        expS = exp(scores/8) via ACT (causal mask added in PSUM beforehand)
        out^T [64hd, 512q] += [V | ones].T @ expS   (denominator rides as row 64)
    normalization deferred + batched: denominators gathered per q-chunk into an
    [8, 512] tile (ACT copies), one reciprocal, K=1 PE broadcast, DVE multiply
    partial = y^T.T @ W_proj[group rows]  -> [2048, 1024]
Host: out[b] = partial(core 2b) + partial(core 2b+1) + b_proj + b_attn_v @ W_proj.
q/k biases are applied on-device (per-partition ACT bias); the v bias commutes
through softmax (rows sum to 1) so its projection is added on the host.
"""
import sys
if '/opt/trn_rl_repo' not in sys.path:
    sys.path.insert(0, '/opt/trn_rl_repo')

import numpy as np
import concourse.bass as bass
import concourse.mybir as mybir
import concourse.tile as tile
from concourse import bacc
from concourse import bass_utils

F32 = mybir.dt.float32
F32R = mybir.dt.float32r
AF = mybir.ActivationFunctionType
ALU = mybir.AluOpType

B, S, D, H, HD = 4, 2048, 1024, 16, 64
NCORES = 8
FPC = 512            # feature dims per core (8 heads * 64)
NPAIR = 4            # head pairs per core
DC = D // 128        # 8 contraction chunks for QKV/proj of x
NST = S // 128       # 16 s-tiles
MASKV = -30000.0     # exp(-30000/8) == 0 in fp32

_CACHE = {}


def _build_program():
    nc = bacc.Bacc("TRN2", target_bir_lowering=False, debug=False,
                   enable_asserts=False, num_devices=NCORES)

    xT_d = nc.dram_tensor("xT", [D, S], F32R, kind="ExternalInput").ap()
    wq_d = nc.dram_tensor("wq", [D, FPC], F32R, kind="ExternalInput").ap()
    wk_d = nc.dram_tensor("wk", [D, FPC], F32R, kind="ExternalInput").ap()
    wv_d = nc.dram_tensor("wv", [D, FPC], F32R, kind="ExternalInput").ap()
    wp_d = nc.dram_tensor("wp", [FPC, D], F32R, kind="ExternalInput").ap()
    bq_d = nc.dram_tensor("bq", [FPC], F32, kind="ExternalInput").ap()
    bk_d = nc.dram_tensor("bk", [FPC], F32, kind="ExternalInput").ap()
    out_d = nc.dram_tensor("out", [S, D], F32, kind="ExternalOutput").ap()

    from contextlib import ExitStack
    with tile.TileContext(nc) as tc, ExitStack() as ctx:
        persist = ctx.enter_context(tc.tile_pool(name="persist", bufs=1))
        QT = [persist.tile([128, S], F32R, name=f"qt{p}") for p in range(NPAIR)]
        KT = [persist.tile([128, S], F32R, name=f"kt{p}") for p in range(NPAIR)]
        # V tiles: [128 s, 8 heads, 65] -- col 64 is the ones column (denominator)
        Vt = [persist.tile([128, 8, 65], F32R, name=f"v{i}") for i in range(NST)]

        # ---------------- Phase 1: Q^T, K^T, V ----------------
        with ExitStack() as c1:
            wpool = c1.enter_context(tc.tile_pool(name="wpool", bufs=1))
            xpool = c1.enter_context(tc.tile_pool(name="xpool", bufs=2))
            p1ps = c1.enter_context(tc.tile_pool(name="p1ps", bufs=6, space="PSUM"))

            wq_sb = wpool.tile([128, DC, FPC], F32R, name="wq_sb")
            wk_sb = wpool.tile([128, DC, FPC], F32R, name="wk_sb")
            wv_sb = wpool.tile([128, DC, FPC], F32R, name="wv_sb")
            # per-chunk DMAs spread across queues (faster preamble)
            for c in range(DC):
                cs = slice(128 * c, 128 * c + 128)
                nc.sync.dma_start(wq_sb[:, c, :], wq_d[cs, :])
                nc.sync.dma_start(wk_sb[:, c, :], wk_d[cs, :])
                nc.sync.dma_start(wv_sb[:, c, :], wv_d[cs, :])
            bq_sb = wpool.tile([128, 4], F32, name="bq_sb")
            bk_sb = wpool.tile([128, 4], F32, name="bk_sb")
            nc.sync.dma_start(bq_sb[:], bq_d.rearrange("(c p) -> p c", p=128))
            nc.sync.dma_start(bk_sb[:], bk_d.rearrange("(c p) -> p c", p=128))

            onesv_f = wpool.tile([128, 8], F32, name="onesv_f")
            nc.gpsimd.memset(onesv_f[:], 1.0)
            for i in range(NST):
                nc.vector.tensor_copy(Vt[i][:, :, 64], onesv_f[:])

            for qtr in range(4):        # s-chunks of 512
                s0 = 512 * qtr
                xq = xpool.tile([128, DC, 512], F32R, name=f"xq{qtr}", tag="xq")
                for c in range(DC):
                    nc.sync.dma_start(xq[:, c, :],
                                      xT_d[128 * c:128 * c + 128, s0:s0 + 512])
                for f in range(4):      # feature chunks of 128 (= head pair)
                    for nm, w_sb, b_sb, dstT in (("q", wq_sb, bq_sb, QT),
                                                 ("k", wk_sb, bk_sb, KT)):
                        ps = p1ps.tile([128, 512], F32, name=f"ps{nm}{qtr}_{f}",
                                       tag="p1ps")
                        for c in range(DC):
                            nc.tensor.matmul(ps[:],
                                             w_sb[:, c, 128 * f:128 * f + 128],
                                             xq[:, c, :],
                                             start=(c == 0), stop=(c == DC - 1))
                        # psum->sbuf + per-feature bias on the idle ACT engine
                        nc.scalar.activation(dstT[f][:, s0:s0 + 512], ps[:],
                                             AF.Identity, bias=b_sb[:, f:f + 1])
                for ii in range(4):     # V s-tiles of 128 in this quarter
                    i = 4 * qtr + ii
                    psv = p1ps.tile([128, 512], F32, name=f"psv{i}", tag="p1ps")
                    for c in range(DC):
                        nc.tensor.matmul(psv[:],
                                         xq[:, c, 128 * ii:128 * ii + 128],
                                         wv_sb[:, c, :],
                                         start=(c == 0), stop=(c == DC - 1))
                    nc.vector.tensor_copy(
                        Vt[i][:, :, 0:64],
                        psv[:].rearrange("p (h u) -> p h u", h=8))

        # ---------------- Phase 2: attention + projection ----------------
        with ExitStack() as c2:
            per2 = c2.enter_context(tc.tile_pool(name="per2", bufs=1))
            expool = c2.enter_context(tc.tile_pool(name="expool", bufs=3))
            smpool = c2.enter_context(tc.tile_pool(name="smpool", bufs=2))
            outsb = c2.enter_context(tc.tile_pool(name="outsb", bufs=3))
            scps = c2.enter_context(tc.tile_pool(name="scps", bufs=3, space="PSUM"))
            pvps = c2.enter_context(tc.tile_pool(name="pvps", bufs=1, space="PSUM"))

            yT = [per2.tile([128, S], F32R, name=f"yt{p}") for p in range(NPAIR)]
            wp_sb = per2.tile([128, 4, D], F32R, name="wp_sb")
            for c in range(4):
                nc.sync.dma_start(wp_sb[:, c, :], wp_d[128 * c:128 * c + 128, :])

            # multiplicative causal masks [128k, 1024] (A|B halves identical)
            # for 4 diagonal offsets: 1.0 iff kp + 128*oi <= qf, else 0.0.
            # Applied to exp output in SBUF (keeps DVE off the PSUM path).
            masks = [per2.tile([128, 1024], F32R, name=f"mask{oi}")
                     for oi in range(4)]
            mask_f = per2.tile([128, 1024], F32, name="mask_f")
            for oi in range(4):
                nc.gpsimd.memset(mask_f[:], 1.0)
                for half in range(2):
                    nc.gpsimd.affine_select(
                        out=mask_f[:, 512 * half:512 * half + 512],
                        in_=mask_f[:, 512 * half:512 * half + 512],
                        compare_op=ALU.is_ge, fill=0.0,
                        base=-128 * oi, pattern=[[1, 512]], channel_multiplier=-1)
                nc.vector.tensor_copy(masks[oi][:], mask_f[:])

            ones64_f = per2.tile([1, 64], F32, name="ones64_f")
            nc.gpsimd.memset(ones64_f[:], 1.0)
            ones64 = per2.tile([1, 64], F32R, name="ones64")
            nc.vector.tensor_copy(ones64[:], ones64_f[:])

            for j in range(4):          # q-chunks of 512
                q0 = 512 * j
                nk = 4 * (j + 1)
                for p in range(NPAIR):
                    accA = pvps.tile([65, 512], F32, name=f"accA{j}_{p}", tag="accA")
                    accB = pvps.tile([65, 512], F32, name=f"accB{j}_{p}", tag="accB")
                    for t in range(nk):
                        k0 = 128 * t
                        sc = scps.tile([128, 1024], F32, name=f"sc{j}_{p}_{t}",
                                       tag="sc")
                        nc.tensor.matmul(sc[:, 0:512],
                                         KT[p][0:64, k0:k0 + 128],
                                         QT[p][0:64, q0:q0 + 512],
                                         start=True, stop=True)
                        nc.tensor.matmul(sc[:, 512:1024],
                                         KT[p][64:128, k0:k0 + 128],
                                         QT[p][64:128, q0:q0 + 512],
                                         start=True, stop=True)
                        ex = expool.tile([128, 1024], F32R, name=f"ex{j}_{p}_{t}",
                                         tag="ex")
                        nc.scalar.activation(ex[:], sc[:], AF.Exp, scale=0.125)
                        oi = t - 4 * j
                        if oi >= 0:     # diagonal tile -> zero invalid exp entries
                            nc.vector.tensor_tensor(ex[:], ex[:],
                                                    masks[oi][:], ALU.mult)
                        nc.tensor.matmul(accA[:], Vt[t][:, 2 * p, :],
                                         ex[:, 0:512],
                                         start=(t == 0), stop=(t == nk - 1))
                        nc.tensor.matmul(accB[:], Vt[t][:, 2 * p + 1, :],
                                         ex[:, 512:1024],
                                         start=(t == 0), stop=(t == nk - 1))
                    # normalize: fast reciprocal of denom row, K=1 PE broadcast,
                    # then in-place multiply of the unnormalized yT copy
                    for hi, acc in ((0, accA), (1, accB)):
                        r = 2 * p + hi
                        ys = yT[p][64 * hi:64 * hi + 64, q0:q0 + 512]
                        nc.vector.tensor_copy(ys, acc[0:64, :])
                        den = smpool.tile([1, 512], F32, name=f"den{j}_{r}",
                                          tag="den")
                        nc.scalar.copy(den[:], acc[64:65, :])
                        scr = smpool.tile([1, 512], F32, name=f"scr{j}_{r}",
                                          tag="scr")
                        rec = smpool.tile([1, 512], F32, name=f"rec{j}_{r}",
                                          tag="rec")
                        nc.vector.reciprocal_approx_accurate(
                            rec[:], den[:], scr[:])
                        rec_r = smpool.tile([1, 512], F32R, name=f"recr{j}_{r}",
                                            tag="rec_r")
                        nc.vector.tensor_copy(rec_r[:], rec[:])
                        bc = scps.tile([128, 1024], F32, name=f"bc{j}_{r}", tag="sc")
                        nc.tensor.matmul(bc[0:64, 0:512], ones64[:],
                                         rec_r[:], start=True, stop=True)
                        nc.vector.tensor_tensor(ys, bc[0:64, 0:512], ys, ALU.mult)
                # projection for this q-chunk (all pairs' yT just completed)
                for ii in range(4):
                    i = 4 * j + ii
                    for o in range(2):
                        po = scps.tile([128, 1024], F32, name=f"po{i}_{o}",
                                       tag="sc")
                        for p2 in range(NPAIR):
                            nc.tensor.matmul(po[:, 0:512],
                                             yT[p2][:, 128 * i:128 * i + 128],
                                             wp_sb[:, p2, 512 * o:512 * o + 512],
                                             start=(p2 == 0), stop=(p2 == 3))
                        ot = outsb.tile([128, 512], F32, name=f"ot{i}_{o}", tag="ot")
                        nc.vector.tensor_copy(ot[:], po[:, 0:512])
                        nc.sync.dma_start(
                            out_d[128 * i:128 * i + 128, 512 * o:512 * o + 512],
                            ot[:])

    nc.compile()
    return nc


def _get_program():
    if "nc" not in _CACHE:
        _CACHE["nc"] = _build_program()
    return _CACHE["nc"]


def kernel(x, W_attn, b_attn, W_proj, b_proj, _trace=False, _trace_cores=None):
    x = np.asarray(x, np.float32)
    W_attn = np.asarray(W_attn, np.float32)
    b_attn = np.asarray(b_attn, np.float32)
    W_proj = np.asarray(W_proj, np.float32)
    b_proj = np.asarray(b_proj, np.float32)

    nc = _get_program()

    in_maps = []
    for c in range(NCORES):
        b, g = divmod(c, 2)
        gc = slice(FPC * g, FPC * g + FPC)
        in_maps.append({
            "xT": np.ascontiguousarray(x[b].T),
            "wq": np.ascontiguousarray(W_attn[:, 0 * D:1 * D][:, gc]),
            "wk": np.ascontiguousarray(W_attn[:, 1 * D:2 * D][:, gc]),
            "wv": np.ascontiguousarray(W_attn[:, 2 * D:3 * D][:, gc]),
            "wp": np.ascontiguousarray(W_proj[gc, :]),
            "bq": np.ascontiguousarray(b_attn[0 * D:1 * D][gc]),
            "bk": np.ascontiguousarray(b_attn[1 * D:2 * D][gc]),
        })

    kw = {}
    if _trace:
        kw = dict(trace=True, trace_cores=_trace_cores or [0])
    res = bass_utils.run_bass_kernel_spmd(nc, in_maps, core_ids=list(range(NCORES)),
                                          **kw)

    # host-side reduction: v-bias commutes through softmax -> fold via W_proj
    corr = b_proj + b_attn[2 * D:3 * D] @ W_proj
    out = np.empty((B, S, D), np.float32)
    for b in range(B):
        out[b] = res.results[2 * b]["out"] + res.results[2 * b + 1]["out"] + corr

    if _trace:
        kernel._last_results = res
    return out
